# revision 21
# baseline (speedup 1.0000x reference)
"""Trainium2 Bass kernel: causal multi-head attention with RoPE.

Problem: B=2, S=2048, D=1024, H=16 heads, hd=64, fp32.
Sharding: 4-way head-tensor-parallel x 2-way batch-data-parallel over 8 cores.
Each core handles one batch element and 4 heads (256 of the 1024 model dims),
computes its partial contribution to the output projection, and the host sums
the 4 partials per batch element.

Per-core pipeline (bf16 matmul operands, fp32 PSUM accumulation):
  - x arrives as one bf16 tensor; DMA-xbar transposes it in per-512-column
    chunks so the first projection matmul starts ~6us in, with weight DMAs
    running concurrently on other queues.
  - Q/K use a single projection each. The host permutes W rows so each head's
    dims land as [even-dims (32) | odd-dims (32)]; RoPE is then 5 DVE ops per
    chunk using partition-offset operands (T1=x*cos, T2=x*sin, then
    y1 = T1[p] - T2[p+32] / y2 = T2[p-32] + T1[p] written via 3 region ops).
  - scores^T = k_rot^T.T @ q_rot^T per (head-pair, q-chunk, k-block-pair) with
    2x row-tiling (K=64); causal block skip plus column-suffix restriction on
    the 4 diagonal blocks (only q >= k-block-start columns are computed).
  - exp on ACT (scale=1/8 fused) over alive column ranges only; the four
    per-block [128,128] causal triangles are masked multiplicatively on DVE.
  - P^T @ v via matmul with a ones-column appended to v (M=65) so the softmax
    denominator accumulates free in PSUM row 64.
  - 1/den via DVE reciprocal_approx_fast (no ACT table switches); broadcast
    across partitions with K=1 matmuls; normalize PV output straight out of
    PSUM on DVE.
  - y chunk = O_norm @ Wo_slice.T per q-chunk, DMA'd out immediately
    (qc-outer loop: attention, normalize, output projection all interleave).
"""
import numpy as np
import ml_dtypes
from contextlib import ExitStack

import concourse.bass as bass
import concourse.tile as tile
from concourse import bacc, mybir
from concourse.bass_utils import run_bass_kernel_spmd

F32 = mybir.dt.float32
F32R = mybir.dt.float32r
BF16 = mybir.dt.bfloat16

B, S, D, H, HD = 2, 2048, 1024, 16, 64
NCORES = 8
TPG = 4            # head-TP degree (groups of 4 heads)
LH = H // TPG      # 4 local heads per core
LD = LH * HD       # 256 local dims
ROPE_BASE = 10000.0
QC = 512           # q chunk (matmul moving dim)
NQC = S // QC      # 4
NST = S // 128     # 16 s tiles
NDT = D // 128     # 8 d tiles

Exp = mybir.ActivationFunctionType.Exp

_NC_CACHE = None
DEBUG = False


def _build():
    nc = bacc.Bacc("TRN2", target_bir_lowering=False, debug=False,
                   enable_asserts=True, num_devices=NCORES)

    xbf = nc.dram_tensor("xbf", [NDT, S, 128], BF16, kind="ExternalInput").ap()
    wqt = nc.dram_tensor("wqt", [128, 2048], BF16, kind="ExternalInput").ap()
    wkt = nc.dram_tensor("wkt", [128, 2048], BF16, kind="ExternalInput").ap()
    wvt = nc.dram_tensor("wvt", [128, 2048], BF16, kind="ExternalInput").ap()
    wot = nc.dram_tensor("wot", [2, 128, D], BF16, kind="ExternalInput").ap()
    cosd = nc.dram_tensor("cosd", [128, S], BF16, kind="ExternalInput").ap()
    sind = nc.dram_tensor("sind", [128, S], BF16, kind="ExternalInput").ap()
    triud = nc.dram_tensor("triud", [128, 128], BF16,
                           kind="ExternalInput").ap()
    y = nc.dram_tensor("y", [S, D], F32, kind="ExternalOutput").ap()
    dbg = (nc.dram_tensor("dbg", [1024, 1024], F32, kind="ExternalOutput").ap()
           if DEBUG else None)

    with tile.TileContext(nc) as tc, ExitStack() as octx:
        # ---- persistent pools ----
        pers = octx.enter_context(tc.tile_pool(name="pers", bufs=1))
        qkp = octx.enter_context(tc.tile_pool(name="qkp", bufs=1))
        vp = octx.enter_context(tc.tile_pool(name="vp", bufs=1))

        # q/k rotated, per head-pair tile: rows = [hA: y1(32)|y2(32) | hB:...]
        qrot = [qkp.tile([128, S], BF16, tag=f"qrot{j}", name=f"qrot{j}")
                for j in range(2)]
        krot = [qkp.tile([128, S], BF16, tag=f"krot{j}", name=f"krot{j}")
                for j in range(2)]
        # v natural with per-head ones column: cols lh*65..lh*65+64 = v head
        # lh, col lh*65+64 = 1.0
        vsb = [vp.tile([128, 260], BF16, tag=f"v{st}", name=f"v{st}")
               for st in range(NST)]

        # weights / rope tables / mask: scalar (ACT HWDGE) queue, in the
        # order phase P consumes them; x transposes stream on the sync
        # queue concurrently.
        wq = pers.tile([128, 2048], BF16, tag="wq", name="wq")
        nc.scalar.dma_start(wq[:], wqt)
        cos_sb = pers.tile([128, S], BF16, tag="cos")
        nc.scalar.dma_start(cos_sb[:], cosd)
        sin_sb = pers.tile([128, S], BF16, tag="sin")
        nc.scalar.dma_start(sin_sb[:], sind)
        wk = pers.tile([128, 2048], BF16, tag="wk", name="wk")
        nc.scalar.dma_start(wk[:], wkt)
        wv = pers.tile([128, 2048], BF16, tag="wv", name="wv")
        nc.scalar.dma_start(wv[:], wvt)
        triu = pers.tile([128, 128], BF16, tag="triu")
        nc.scalar.dma_start(triu[:], triud)
        wo_sb = []
        for hp in range(2):
            w = pers.tile([128, D], BF16, tag=f"wo{hp}", name=f"wo{hp}")
            nc.scalar.dma_start(w[:], wot[hp])
            wo_sb.append(w)

        xtp = octx.enter_context(tc.tile_pool(name="xtp", bufs=1))
        xt = [xtp.tile([128, S], BF16, tag=f"xt{dt}", name=f"xt{dt}")
              for dt in range(NDT)]

        # ---- phase P: projections + RoPE, chunked by 512-col s-chunks ----
        with ExitStack() as s1:
            s1p = s1.enter_context(tc.tile_pool(name="s1p", bufs=1))
            pqs = s1.enter_context(tc.tile_pool(name="pqs", bufs=3,
                                                space="PSUM"))
            pvs = s1.enter_context(tc.tile_pool(name="pvs", bufs=2,
                                                space="PSUM"))

            for st in range(NST):
                vdst = vsb[st].rearrange("p (h c) -> p h c", c=65)[:, :, 64:65]
                nc.vector.memset(vdst, 1.0)

            for sc in range(NQC):
                c0, c1 = sc * QC, (sc + 1) * QC
                for dt in range(NDT):
                    nc.sync.dma_start_transpose(
                        xt[dt][:, c0:c1], xbf[dt][c0:c1, :])

                # q/k projection + RoPE for this s-chunk
                for (w, rot) in ((wq, qrot), (wk, krot)):
                    for jt in range(2):
                        pq = pqs.tile([128, QC], F32, tag="pq", name="pq")
                        for dt in range(NDT):
                            nc.tensor.matmul(
                                pq[:],
                                w[:, dt * 256 + jt * 128:
                                  dt * 256 + jt * 128 + 128],
                                xt[dt][:, c0:c1],
                                start=(dt == 0), stop=(dt == NDT - 1))
                        xb = s1p.tile([128, QC], BF16, tag="xb", bufs=3,
                                      name="xb")
                        nc.scalar.copy(xb[:], pq[:])
                        t1 = s1p.tile([128, QC], BF16, tag="t1", bufs=2,
                                      name="t1")
                        nc.vector.tensor_mul(t1[:], xb[:], cos_sb[:, c0:c1])
                        # t2s[p] = x[swap32(p)] * sneg[swap32(p)] where
                        # swap32 flips adjacent 32-row blocks and sneg has
                        # -sin on the x2 rows; then rot = t1 + t2s gives
                        # y1 = x1*c - x2*s, y2 = x2*c + x1*s in one add.
                        # (DVE in-operands must share a base partition;
                        # only the out base may shift.)
                        t2 = s1p.tile([128, QC], BF16, tag="t2", bufs=2,
                                      name="t2")
                        for blk in range(4):
                            a, b = blk * 32, (blk ^ 1) * 32
                            nc.vector.tensor_mul(t2[a:a + 32, :],
                                                 xb[b:b + 32, :],
                                                 sin_sb[b:b + 32, c0:c1])
                        nc.vector.tensor_add(rot[jt][:, c0:c1], t1[:], t2[:])
                        if DEBUG and w is wq and jt == 0 and sc == 0:
                            nc.gpsimd.dma_start(dbg[260:388, 0:512], xb[:])
                            nc.gpsimd.dma_start(dbg[644:772, 0:512],
                                                rot[0][:, 0:512])

                # v projection for the 4 s-tiles of this chunk
                for st in range(4 * sc, 4 * sc + 4):
                    pv = pvs.tile([128, 256], F32, tag="pv", name="pv")
                    for dt in range(NDT):
                        nc.tensor.matmul(
                            pv[:],
                            xt[dt][:, st * 128:(st + 1) * 128],
                            wv[:, dt * 256:(dt + 1) * 256],
                            start=(dt == 0), stop=(dt == NDT - 1))
                    dst = vsb[st].rearrange("p (h c) -> p h c",
                                            c=65)[:, :, 0:64]
                    src = pv.rearrange("p (h c) -> p h c", c=64)
                    nc.scalar.copy(dst, src)
                    if DEBUG and st == 0:
                        nc.gpsimd.dma_start(dbg[388:516, 0:260],
                                            vsb[0][:, 0:260])

        # ---- attention + normalize + output projection, qc-outer ----
        with ExitStack() as s2:
            s2p = s2.enter_context(tc.tile_pool(name="s2p", bufs=1))
            pss = s2.enter_context(tc.tile_pool(name="pss", bufs=2,
                                                space="PSUM"))
            pso = s2.enter_context(tc.tile_pool(name="pso", bufs=2,
                                                space="PSUM"))
            psy = s2.enter_context(tc.tile_pool(name="psy", bufs=2,
                                                space="PSUM"))

            pend_y = [None]

            def emit_y(qc, otn):
                for j in range(4):
                    st = 4 * qc + j
                    ysb = s2p.tile([128, D], F32, tag="ysb", bufs=3,
                                   name="ysb")
                    for mc in range(2):
                        py = psy.tile([128, 512], F32, tag="py", name="py")
                        for hp in range(2):
                            nc.tensor.matmul(
                                py[:],
                                otn[hp][:, j * 128:(j + 1) * 128],
                                wo_sb[hp][:, mc * 512:(mc + 1) * 512],
                                start=(hp == 0), stop=(hp == 1))
                        dst = ysb[:, mc * 512:(mc + 1) * 512]
                        if mc == 0:
                            nc.scalar.copy(dst, py[:])
                        else:
                            nc.vector.tensor_copy(dst, py[:])
                    nc.sync.dma_start(y[st * 128:(st + 1) * 128, :], ysb[:])

            for qc in range(NQC):
                npair = 2 * qc + 2
                otn = [s2p.tile([128, QC], BF16, tag=f"otn{hp}", bufs=2,
                                name=f"otn{hp}") for hp in range(2)]
                for hp in range(2):          # head pair = (2hp, 2hp+1)
                    if hp == 1 and pend_y[0] is not None:
                        # previous qc's output projection: otn is long since
                        # final, and hp0's scores above kept ACT fed
                        emit_y(qc - 1, pend_y[0])
                        pend_y[0] = None
                    po = [pso.tile([128, QC], F32, tag="po", name="po")
                          for _ in range(2)]

                    def emit_pv(kp, pts, mlist, hp=hp, qc=qc, po=po):
                        for z in range(2):
                            lh = 2 * hp + z
                            for e in range(2):
                                kb = 2 * kp + e
                                m = mlist[e]
                                nc.tensor.matmul(
                                    po[z][0:65, m:QC],
                                    vsb[kb][:, lh * 65:lh * 65 + 65],
                                    pts[z][:, e * QC + m:(e + 1) * QC],
                                    start=(kb == 0),
                                    stop=(kb == 4 * qc + 3))

                    pend = []
                    for kp in range(npair):
                        diag = kp >= npair - 2
                        # alive-column start within the qc block, per e
                        mlist = []
                        for e in range(2):
                            kb = 2 * kp + e
                            r = kb - 4 * qc
                            mlist.append(128 * r if diag else 0)
                        pts = []
                        for z in range(2):   # z=0: rows 0:64, z=1: 64:128
                            r0 = 64 * z
                            ps_ = pss.tile([128, 1024], F32, tag="ps",
                                           name="ps_")
                            for e in range(2):
                                kb = 2 * kp + e
                                m = mlist[e]
                                nc.tensor.matmul(
                                    ps_[:, e * QC + m:(e + 1) * QC],
                                    krot[hp][r0:r0 + 64,
                                             kb * 128:(kb + 1) * 128],
                                    qrot[hp][r0:r0 + 64,
                                             qc * QC + m:(qc + 1) * QC],
                                    start=True, stop=True,
                                    tile_position=(r0, 0))
                            pt = s2p.tile([128, 1024], BF16, tag="pt",
                                          bufs=6, name="pt")
                            if not diag:
                                nc.scalar.activation(pt[:], ps_[:], Exp,
                                                     scale=0.125)
                            else:
                                for e in range(2):
                                    m = mlist[e]
                                    nc.scalar.activation(
                                        pt[:, e * QC + m:(e + 1) * QC],
                                        ps_[:, e * QC + m:(e + 1) * QC],
                                        Exp, scale=0.125)
                                    # causal triangle on the first 128 alive
                                    # columns of each diagonal block
                                    nc.vector.tensor_mul(
                                        pt[:, e * QC + m:e * QC + m + 128],
                                        pt[:, e * QC + m:e * QC + m + 128],
                                        triu[:])
                            if (DEBUG and qc == 0 and hp == 0 and kp == 0
                                    and z == 0):
                                nc.gpsimd.dma_start(dbg[516:644, 0:1024],
                                                    pt[:])
                            pts.append(pt)
                        # software-pipeline: PV lags two rounds behind the
                        # scores so the PE never waits on exp/mask latency.
                        pend.append((kp, pts, mlist))
                        if len(pend) > 2:
                            emit_pv(*pend.pop(0))
                    for args in pend:
                        emit_pv(*args)

                    # stage O^T+den out of PSUM fast (frees po for the next
                    # head pair's PV), then denominator reciprocal ->
                    # partition broadcast -> normalized O^T (bf16).
                    # (partition_broadcast requires an out AP based at
                    # partition 0, hence per-z [64, QC] tiles)
                    for z in range(2):
                        otu = s2p.tile([64, QC], F32, tag="otu", bufs=4,
                                       name="otu")
                        nc.vector.tensor_copy(otu[:], po[z][0:64, :])
                        dsb = s2p.tile([1, QC], F32, tag="dsb", bufs=2,
                                       name="dsb")
                        nc.vector.tensor_copy(dsb[:], po[z][64:65, :])
                        r = s2p.tile([1, QC], F32, tag="rsb", bufs=4,
                                     name="rsb")
                        nc.vector.reciprocal_approx_fast(
                            out=r[:], in_=dsb[:])
                        pbs = s2p.tile([64, QC], F32, tag="pbs", bufs=4,
                                       name="pbs")
                        nc.gpsimd.partition_broadcast(pbs[:], r[:])
                        nc.vector.tensor_mul(otn[hp][64 * z:64 * z + 64, :],
                                             otu[:], pbs[:])
                        if DEBUG and qc == 0 and hp == 0:
                            nc.sync.dma_start(dbg[z:z + 1, 0:512], r[:])
                            nc.sync.dma_start(dbg[4 + 64 * z:4 + 64 * z + 64,
                                                  0:512], pbs[:])
                    if DEBUG and qc == 0 and hp == 0:
                        nc.gpsimd.dma_start(dbg[132:260, 0:512],
                                            otn[0][:])
                pend_y[0] = otn
            emit_y(NQC - 1, pend_y[0])

    nc.compile()
    return nc


def _get_nc():
    global _NC_CACHE
    if _NC_CACHE is None:
        _NC_CACHE = _build()
    return _NC_CACHE


def _host_prep(x, Wq, Wk, Wv, Wo):
    """Build the 8 per-core input maps."""
    bf = ml_dtypes.bfloat16
    x = np.asarray(x, dtype=np.float32)
    Wq, Wk, Wv, Wo = (np.asarray(w, dtype=np.float32) for w in (Wq, Wk, Wv, Wo))

    def tile128(wt):  # [1024, 256] -> [128, 2048] with d-tiles along free dim
        return np.ascontiguousarray(
            wt.reshape(NDT, 128, LD).transpose(1, 0, 2).reshape(
                128, NDT * LD).astype(bf))

    def perm_qk(W, g):
        # rows per jt tile: [h0 evens(32) | h0 odds(32) | h1 evens | h1 odds]
        blocks = []
        for lh in range(LH):
            gh = g * LH + lh
            O = W[gh * HD:(gh + 1) * HD]          # [64, 1024]
            blocks.append(O[0::2])
            blocks.append(O[1::2])
        Wp = np.concatenate(blocks, axis=0)       # [256, 1024]
        return tile128(Wp.T)

    t = np.arange(32, dtype=np.float64)
    theta = 1.0 / (ROPE_BASE ** (2.0 * t / HD))
    ang = np.arange(S, dtype=np.float64)[:, None] * theta[None, :]  # [S, 32]
    c32 = np.cos(ang).T.astype(np.float32)        # [32, S]
    s32 = np.sin(ang).T.astype(np.float32)
    cosd = np.ascontiguousarray(np.tile(c32, (4, 1)).astype(bf))  # [128, S]
    sneg = np.tile(s32, (4, 1))
    sneg[32:64] *= -1.0   # x2 rows carry -sin so rot = x*cos + swap(x*sneg)
    sneg[96:128] *= -1.0
    sind = np.ascontiguousarray(sneg.astype(bf))

    p = np.arange(128)[:, None]
    c = np.arange(128)[None, :]
    triud = np.ascontiguousarray((p <= c).astype(bf))

    per_b = []
    for b in range(B):
        xb = x[b].astype(bf)
        xb = xb.reshape(S, NDT, 128).transpose(1, 0, 2)
        per_b.append(np.ascontiguousarray(xb))

    per_g = []
    for g in range(TPG):
        wq = perm_qk(Wq, g)
        wk = perm_qk(Wk, g)
        wvt_ = tile128(Wv[g * LD:(g + 1) * LD].T)
        wot_ = np.ascontiguousarray(
            Wo[:, g * LD:(g + 1) * LD].T.reshape(2, 128, D).astype(bf))
        per_g.append((wq, wk, wvt_, wot_))

    in_maps = []
    for core in range(NCORES):
        b, g = divmod(core, TPG)
        wq, wk, wvt_, wot_ = per_g[g]
        in_maps.append({
            "xbf": per_b[b],
            "wqt": wq, "wkt": wk, "wvt": wvt_, "wot": wot_,
            "cosd": cosd, "sind": sind, "triud": triud,
        })
    return in_maps


def run(inputs, trace=False):
    """Run on all 8 cores; returns (y_full, BassKernelResults)."""
    x = inputs["x"]
    in_maps = _host_prep(x, inputs["Wq"], inputs["Wk"], inputs["Wv"],
                         inputs["Wo"])
    nc = _get_nc()
    kw = {}
    if trace:
        kw = dict(trace=True, trace_cores=[0])
    res = run_bass_kernel_spmd(nc, in_maps, core_ids=list(range(NCORES)), **kw)
    y = np.zeros((B, S, D), dtype=np.float32)
    for c in range(NCORES):
        y[c // TPG] += res.results[c]["y"]
    return y, res


def kernel(x, Wq, Wk, Wv, Wo, n_heads):
    assert int(n_heads) == H
    y, _ = run({"x": x, "Wq": Wq, "Wk": Wk, "Wv": Wv, "Wo": Wo})
    return y


# revision 22
# speedup vs baseline: 1.1275x; 1.1275x over previous
"""Trainium2 Bass kernel: causal multi-head attention with RoPE.

Problem: B=2, S=2048, D=1024, H=16 heads, hd=64, fp32.
Sharding: 4-way head-tensor-parallel x 2-way batch-data-parallel over 8 cores.
Each core handles one batch element and 4 heads (256 of the 1024 model dims),
computes its partial contribution to the output projection, and the host sums
the 4 partials per batch element.

Per-core pipeline (bf16 matmul operands, fp32 PSUM accumulation):
  - x arrives as one bf16 tensor; DMA-xbar transposes it in per-512-column
    chunks so the first projection matmul starts ~6us in, with weight DMAs
    running concurrently on other queues.
  - Q/K use a single projection each. The host permutes W rows so each head's
    dims land as [even-dims (32) | odd-dims (32)]; RoPE is then 5 DVE ops per
    chunk using partition-offset operands (T1=x*cos, T2=x*sin, then
    y1 = T1[p] - T2[p+32] / y2 = T2[p-32] + T1[p] written via 3 region ops).
  - scores^T = k_rot^T.T @ q_rot^T per (head-pair, q-chunk, k-block-pair) with
    2x row-tiling (K=64); causal block skip plus column-suffix restriction on
    the 4 diagonal blocks (only q >= k-block-start columns are computed).
  - exp on ACT (scale=1/8 fused) over alive column ranges only; the four
    per-block [128,128] causal triangles are masked multiplicatively on DVE.
  - P^T @ v via matmul with a ones-column appended to v (M=65) so the softmax
    denominator accumulates free in PSUM row 64.
  - 1/den via DVE reciprocal_approx_fast (no ACT table switches); broadcast
    across partitions with K=1 matmuls; normalize PV output straight out of
    PSUM on DVE.
  - y chunk = O_norm @ Wo_slice.T per q-chunk, DMA'd out immediately
    (qc-outer loop: attention, normalize, output projection all interleave).
"""
import numpy as np
import ml_dtypes
from contextlib import ExitStack

import concourse.bass as bass
import concourse.tile as tile
from concourse import bacc, mybir
from concourse.bass_utils import run_bass_kernel_spmd

F32 = mybir.dt.float32
F32R = mybir.dt.float32r
BF16 = mybir.dt.bfloat16

B, S, D, H, HD = 2, 2048, 1024, 16, 64
NCORES = 8
TPG = 4            # head-TP degree (groups of 4 heads)
LH = H // TPG      # 4 local heads per core
LD = LH * HD       # 256 local dims
ROPE_BASE = 10000.0
QC = 512           # q chunk (matmul moving dim)
NQC = S // QC      # 4
NST = S // 128     # 16 s tiles
NDT = D // 128     # 8 d tiles

Exp = mybir.ActivationFunctionType.Exp

_NC_CACHE = None
DEBUG = False


def _build():
    nc = bacc.Bacc("TRN2", target_bir_lowering=False, debug=False,
                   enable_asserts=True, num_devices=NCORES)

    xbf = nc.dram_tensor("xbf", [NDT, S, 128], BF16, kind="ExternalInput").ap()
    wqt = nc.dram_tensor("wqt", [128, 2048], BF16, kind="ExternalInput").ap()
    wkt = nc.dram_tensor("wkt", [128, 2048], BF16, kind="ExternalInput").ap()
    wvt = nc.dram_tensor("wvt", [128, 2048], BF16, kind="ExternalInput").ap()
    wot = nc.dram_tensor("wot", [2, 128, D], BF16, kind="ExternalInput").ap()
    cosd = nc.dram_tensor("cosd", [128, S], BF16, kind="ExternalInput").ap()
    sind = nc.dram_tensor("sind", [128, S], BF16, kind="ExternalInput").ap()
    triud = nc.dram_tensor("triud", [128, 128], BF16,
                           kind="ExternalInput").ap()
    y = nc.dram_tensor("y", [S, D], F32, kind="ExternalOutput").ap()
    dbg = (nc.dram_tensor("dbg", [1024, 1024], F32, kind="ExternalOutput").ap()
           if DEBUG else None)

    with tile.TileContext(nc) as tc, ExitStack() as octx:
        # ---- persistent pools ----
        pers = octx.enter_context(tc.tile_pool(name="pers", bufs=1))
        qkp = octx.enter_context(tc.tile_pool(name="qkp", bufs=1))
        vp = octx.enter_context(tc.tile_pool(name="vp", bufs=1))

        # q/k rotated, per head-pair tile: rows = [hA: y1(32)|y2(32) | hB:...]
        qrot = [qkp.tile([128, S], BF16, tag=f"qrot{j}", name=f"qrot{j}")
                for j in range(2)]
        krot = [qkp.tile([128, S], BF16, tag=f"krot{j}", name=f"krot{j}")
                for j in range(2)]
        # v natural with per-head ones column: cols lh*65..lh*65+64 = v head
        # lh, col lh*65+64 = 1.0
        vsb = [vp.tile([128, 260], BF16, tag=f"v{st}", name=f"v{st}")
               for st in range(NST)]

        # weights / rope tables / mask: SWDGE (gpsimd) + scalar queues so
        # the sync-queue transpose stream is disturbed as little as
        # possible (Tile serializes DMA_TRANSPOSE against other DMAs).
        wq = pers.tile([128, 2048], BF16, tag="wq", name="wq")
        nc.gpsimd.dma_start(wq[:], wqt)
        wk = pers.tile([128, 2048], BF16, tag="wk", name="wk")
        nc.gpsimd.dma_start(wk[:], wkt)
        wv = pers.tile([128, 2048], BF16, tag="wv", name="wv")
        nc.gpsimd.dma_start(wv[:], wvt)
        wo_sb = []
        for hp in range(2):
            w = pers.tile([128, D], BF16, tag=f"wo{hp}", name=f"wo{hp}")
            nc.gpsimd.dma_start(w[:], wot[hp])
            wo_sb.append(w)
        cos_sb = pers.tile([128, S], BF16, tag="cos")
        nc.scalar.dma_start(cos_sb[:], cosd)
        sin_sb = pers.tile([128, S], BF16, tag="sin")
        nc.scalar.dma_start(sin_sb[:], sind)
        triu = pers.tile([128, 128], BF16, tag="triu")
        nc.scalar.dma_start(triu[:], triud)

        xtp = octx.enter_context(tc.tile_pool(name="xtp", bufs=1))
        xt = [xtp.tile([128, S], BF16, tag=f"xt{dt}", name=f"xt{dt}")
              for dt in range(NDT)]

        # ---- phase P: projections + RoPE, chunked by 512-col s-chunks ----
        with ExitStack() as s1:
            s1p = s1.enter_context(tc.tile_pool(name="s1p", bufs=1))
            pqs = s1.enter_context(tc.tile_pool(name="pqs", bufs=3,
                                                space="PSUM"))
            pvs = s1.enter_context(tc.tile_pool(name="pvs", bufs=2,
                                                space="PSUM"))

            for st in range(NST):
                vdst = vsb[st].rearrange("p (h c) -> p h c", c=65)[:, :, 64:65]
                nc.vector.memset(vdst, 1.0)

            for sc in range(NQC):
                c0, c1 = sc * QC, (sc + 1) * QC
                for dt in range(NDT):
                    nc.sync.dma_start_transpose(
                        xt[dt][:, c0:c1], xbf[dt][c0:c1, :])

                # q/k projection + RoPE for this s-chunk
                for (w, rot) in ((wq, qrot), (wk, krot)):
                    for jt in range(2):
                        pq = pqs.tile([128, QC], F32, tag="pq", name="pq")
                        for dt in range(NDT):
                            nc.tensor.matmul(
                                pq[:],
                                w[:, dt * 256 + jt * 128:
                                  dt * 256 + jt * 128 + 128],
                                xt[dt][:, c0:c1],
                                start=(dt == 0), stop=(dt == NDT - 1))
                        xb = s1p.tile([128, QC], BF16, tag="xb", bufs=3,
                                      name="xb")
                        nc.scalar.copy(xb[:], pq[:])
                        t1 = s1p.tile([128, QC], BF16, tag="t1", bufs=2,
                                      name="t1")
                        nc.vector.tensor_mul(t1[:], xb[:], cos_sb[:, c0:c1])
                        # t2s[p] = x[swap32(p)] * sneg[swap32(p)] where
                        # swap32 flips adjacent 32-row blocks and sneg has
                        # -sin on the x2 rows; then rot = t1 + t2s gives
                        # y1 = x1*c - x2*s, y2 = x2*c + x1*s in one add.
                        # (DVE in-operands must share a base partition;
                        # only the out base may shift.)
                        t2 = s1p.tile([128, QC], BF16, tag="t2", bufs=2,
                                      name="t2")
                        for blk in range(4):
                            a, b = blk * 32, (blk ^ 1) * 32
                            nc.vector.tensor_mul(t2[a:a + 32, :],
                                                 xb[b:b + 32, :],
                                                 sin_sb[b:b + 32, c0:c1])
                        nc.vector.tensor_add(rot[jt][:, c0:c1], t1[:], t2[:])
                        if DEBUG and w is wq and jt == 0 and sc == 0:
                            nc.gpsimd.dma_start(dbg[260:388, 0:512], xb[:])
                            nc.gpsimd.dma_start(dbg[644:772, 0:512],
                                                rot[0][:, 0:512])

                # v projection for the 4 s-tiles of this chunk
                for st in range(4 * sc, 4 * sc + 4):
                    pv = pvs.tile([128, 256], F32, tag="pv", name="pv")
                    for dt in range(NDT):
                        nc.tensor.matmul(
                            pv[:],
                            xt[dt][:, st * 128:(st + 1) * 128],
                            wv[:, dt * 256:(dt + 1) * 256],
                            start=(dt == 0), stop=(dt == NDT - 1))
                    dst = vsb[st].rearrange("p (h c) -> p h c",
                                            c=65)[:, :, 0:64]
                    src = pv.rearrange("p (h c) -> p h c", c=64)
                    nc.scalar.copy(dst, src)
                    if DEBUG and st == 0:
                        nc.gpsimd.dma_start(dbg[388:516, 0:260],
                                            vsb[0][:, 0:260])

        # ---- attention + normalize + output projection, qc-outer ----
        with ExitStack() as s2:
            s2p = s2.enter_context(tc.tile_pool(name="s2p", bufs=1))
            pss = s2.enter_context(tc.tile_pool(name="pss", bufs=2,
                                                space="PSUM"))
            pso = s2.enter_context(tc.tile_pool(name="pso", bufs=2,
                                                space="PSUM"))
            psy = s2.enter_context(tc.tile_pool(name="psy", bufs=2,
                                                space="PSUM"))

            pend_y = [None]

            def emit_y(qc, otn):
                for j in range(4):
                    st = 4 * qc + j
                    ysb = s2p.tile([128, D], F32, tag="ysb", bufs=3,
                                   name="ysb")
                    for mc in range(2):
                        py = psy.tile([128, 512], F32, tag="py", name="py")
                        for hp in range(2):
                            nc.tensor.matmul(
                                py[:],
                                otn[hp][:, j * 128:(j + 1) * 128],
                                wo_sb[hp][:, mc * 512:(mc + 1) * 512],
                                start=(hp == 0), stop=(hp == 1))
                        dst = ysb[:, mc * 512:(mc + 1) * 512]
                        if mc == 0:
                            nc.scalar.copy(dst, py[:])
                        else:
                            nc.vector.tensor_copy(dst, py[:])
                    nc.sync.dma_start(y[st * 128:(st + 1) * 128, :], ysb[:])

            for qc in range(NQC):
                npair = 2 * qc + 2
                otn = [s2p.tile([128, QC], BF16, tag=f"otn{hp}", bufs=2,
                                name=f"otn{hp}") for hp in range(2)]
                for hp in range(2):          # head pair = (2hp, 2hp+1)
                    if hp == 1 and pend_y[0] is not None:
                        # previous qc's output projection: otn is long since
                        # final, and hp0's scores above kept ACT fed
                        emit_y(qc - 1, pend_y[0])
                        pend_y[0] = None
                    po = [pso.tile([128, QC], F32, tag="po", name="po")
                          for _ in range(2)]

                    def emit_pv(kp, pts, mlist, hp=hp, qc=qc, po=po):
                        for z in range(2):
                            lh = 2 * hp + z
                            for e in range(2):
                                kb = 2 * kp + e
                                m = mlist[e]
                                nc.tensor.matmul(
                                    po[z][0:65, m:QC],
                                    vsb[kb][:, lh * 65:lh * 65 + 65],
                                    pts[z][:, e * QC + m:(e + 1) * QC],
                                    start=(kb == 0),
                                    stop=(kb == 4 * qc + 3))

                    pend = []
                    for kp in range(npair):
                        diag = kp >= npair - 2
                        # alive-column start within the qc block, per e
                        mlist = []
                        for e in range(2):
                            kb = 2 * kp + e
                            r = kb - 4 * qc
                            mlist.append(128 * r if diag else 0)
                        pts = []
                        for z in range(2):   # z=0: rows 0:64, z=1: 64:128
                            r0 = 64 * z
                            ps_ = pss.tile([128, 1024], F32, tag="ps",
                                           name="ps_")
                            for e in range(2):
                                kb = 2 * kp + e
                                m = mlist[e]
                                nc.tensor.matmul(
                                    ps_[:, e * QC + m:(e + 1) * QC],
                                    krot[hp][r0:r0 + 64,
                                             kb * 128:(kb + 1) * 128],
                                    qrot[hp][r0:r0 + 64,
                                             qc * QC + m:(qc + 1) * QC],
                                    start=True, stop=True,
                                    tile_position=(r0, 0))
                            pt = s2p.tile([128, 1024], BF16, tag="pt",
                                          bufs=6, name="pt")
                            if not diag:
                                nc.scalar.activation(pt[:], ps_[:], Exp,
                                                     scale=0.125)
                            else:
                                for e in range(2):
                                    m = mlist[e]
                                    nc.scalar.activation(
                                        pt[:, e * QC + m:(e + 1) * QC],
                                        ps_[:, e * QC + m:(e + 1) * QC],
                                        Exp, scale=0.125)
                                    # causal triangle on the first 128 alive
                                    # columns of each diagonal block
                                    nc.vector.tensor_mul(
                                        pt[:, e * QC + m:e * QC + m + 128],
                                        pt[:, e * QC + m:e * QC + m + 128],
                                        triu[:])
                            if (DEBUG and qc == 0 and hp == 0 and kp == 0
                                    and z == 0):
                                nc.gpsimd.dma_start(dbg[516:644, 0:1024],
                                                    pt[:])
                            pts.append(pt)
                        # software-pipeline: PV lags two rounds behind the
                        # scores so the PE never waits on exp/mask latency.
                        pend.append((kp, pts, mlist))
                        if len(pend) > 2:
                            emit_pv(*pend.pop(0))
                    for args in pend:
                        emit_pv(*args)

                    # stage O^T+den out of PSUM fast (frees po for the next
                    # head pair's PV), then denominator reciprocal ->
                    # partition broadcast -> normalized O^T (bf16).
                    # (partition_broadcast requires an out AP based at
                    # partition 0, hence per-z [64, QC] tiles)
                    for z in range(2):
                        otu = s2p.tile([64, QC], F32, tag="otu", bufs=4,
                                       name="otu")
                        nc.vector.tensor_copy(otu[:], po[z][0:64, :])
                        dsb = s2p.tile([1, QC], F32, tag="dsb", bufs=2,
                                       name="dsb")
                        nc.vector.tensor_copy(dsb[:], po[z][64:65, :])
                        r = s2p.tile([1, QC], F32, tag="rsb", bufs=4,
                                     name="rsb")
                        nc.vector.reciprocal_approx_fast(
                            out=r[:], in_=dsb[:])
                        pbs = s2p.tile([64, QC], F32, tag="pbs", bufs=4,
                                       name="pbs")
                        nc.gpsimd.partition_broadcast(pbs[:], r[:])
                        nc.vector.tensor_mul(otn[hp][64 * z:64 * z + 64, :],
                                             otu[:], pbs[:])
                        if DEBUG and qc == 0 and hp == 0:
                            nc.sync.dma_start(dbg[z:z + 1, 0:512], r[:])
                            nc.sync.dma_start(dbg[4 + 64 * z:4 + 64 * z + 64,
                                                  0:512], pbs[:])
                    if DEBUG and qc == 0 and hp == 0:
                        nc.gpsimd.dma_start(dbg[132:260, 0:512],
                                            otn[0][:])
                pend_y[0] = otn
            emit_y(NQC - 1, pend_y[0])

    nc.compile()
    return nc


def _get_nc():
    global _NC_CACHE
    if _NC_CACHE is None:
        _NC_CACHE = _build()
    return _NC_CACHE


def _host_prep(x, Wq, Wk, Wv, Wo):
    """Build the 8 per-core input maps."""
    bf = ml_dtypes.bfloat16
    x = np.asarray(x, dtype=np.float32)
    Wq, Wk, Wv, Wo = (np.asarray(w, dtype=np.float32) for w in (Wq, Wk, Wv, Wo))

    def tile128(wt):  # [1024, 256] -> [128, 2048] with d-tiles along free dim
        return np.ascontiguousarray(
            wt.reshape(NDT, 128, LD).transpose(1, 0, 2).reshape(
                128, NDT * LD).astype(bf))

    def perm_qk(W, g):
        # rows per jt tile: [h0 evens(32) | h0 odds(32) | h1 evens | h1 odds]
        blocks = []
        for lh in range(LH):
            gh = g * LH + lh
            O = W[gh * HD:(gh + 1) * HD]          # [64, 1024]
            blocks.append(O[0::2])
            blocks.append(O[1::2])
        Wp = np.concatenate(blocks, axis=0)       # [256, 1024]
        return tile128(Wp.T)

    t = np.arange(32, dtype=np.float64)
    theta = 1.0 / (ROPE_BASE ** (2.0 * t / HD))
    ang = np.arange(S, dtype=np.float64)[:, None] * theta[None, :]  # [S, 32]
    c32 = np.cos(ang).T.astype(np.float32)        # [32, S]
    s32 = np.sin(ang).T.astype(np.float32)
    cosd = np.ascontiguousarray(np.tile(c32, (4, 1)).astype(bf))  # [128, S]
    sneg = np.tile(s32, (4, 1))
    sneg[32:64] *= -1.0   # x2 rows carry -sin so rot = x*cos + swap(x*sneg)
    sneg[96:128] *= -1.0
    sind = np.ascontiguousarray(sneg.astype(bf))

    p = np.arange(128)[:, None]
    c = np.arange(128)[None, :]
    triud = np.ascontiguousarray((p <= c).astype(bf))

    per_b = []
    for b in range(B):
        xb = x[b].astype(bf)
        xb = xb.reshape(S, NDT, 128).transpose(1, 0, 2)
        per_b.append(np.ascontiguousarray(xb))

    per_g = []
    for g in range(TPG):
        wq = perm_qk(Wq, g)
        wk = perm_qk(Wk, g)
        wvt_ = tile128(Wv[g * LD:(g + 1) * LD].T)
        wot_ = np.ascontiguousarray(
            Wo[:, g * LD:(g + 1) * LD].T.reshape(2, 128, D).astype(bf))
        per_g.append((wq, wk, wvt_, wot_))

    in_maps = []
    for core in range(NCORES):
        b, g = divmod(core, TPG)
        wq, wk, wvt_, wot_ = per_g[g]
        in_maps.append({
            "xbf": per_b[b],
            "wqt": wq, "wkt": wk, "wvt": wvt_, "wot": wot_,
            "cosd": cosd, "sind": sind, "triud": triud,
        })
    return in_maps


def run(inputs, trace=False):
    """Run on all 8 cores; returns (y_full, BassKernelResults)."""
    x = inputs["x"]
    in_maps = _host_prep(x, inputs["Wq"], inputs["Wk"], inputs["Wv"],
                         inputs["Wo"])
    nc = _get_nc()
    kw = {}
    if trace:
        kw = dict(trace=True, trace_cores=[0])
    res = run_bass_kernel_spmd(nc, in_maps, core_ids=list(range(NCORES)), **kw)
    y = np.zeros((B, S, D), dtype=np.float32)
    for c in range(NCORES):
        y[c // TPG] += res.results[c]["y"]
    return y, res


def kernel(x, Wq, Wk, Wv, Wo, n_heads):
    assert int(n_heads) == H
    y, _ = run({"x": x, "Wq": Wq, "Wk": Wk, "Wv": Wv, "Wo": Wo})
    return y
